# revision 46
# baseline (speedup 1.0000x reference)
"""Trainium2 Bass kernel for nn_ConsistencyMaskFromBoxes.

Computes: loss = WEIGHT * mean(BCEWithLogits(seg_pred * eff, boxes_mask * eff))

Algorithm
---------
Per-pixel BCE with a {0,1} target t factorizes:
    bce = softplus(l) - l*t
so  sum(bce) = sum(softplus(l)) - sum_{mask} l.

sum(softplus(l)) uses softplus(l) = -ln(sigmoid(-l)):
  * scalar engine: one Sigmoid pass over both images ([128, 6400] fp8 in,
    bf16 out, 4 blocks of 1600 overlapping the input DMA).
  * vector engine: per block, 3 product-tree levels compress 8 sigmoids
    into one bf16 product p in (0,1), then an X-reduce of the uint16 bit
    pattern J of p: ln p ~ (J - K)*ln2/128 (fast-log bit trick; affine map
    + sawtooth mean correction on host).

sum_{mask} l: host decomposes each image's box union into DISJOINT rects
(sweep line), so mask = sum_r rowhit[r,y]*colhit[r,x] exactly. The row
contraction is PE matmuls accumulated over 5 row-tiles into a shared PSUM
tile (one 32-partition band per (image, chunk)); the colhit dot is one DVE
multiply (PSUM x bf16 colhit -> scr) whose product tile is DMA'd out while
the last block finishes; the host does the mask sum.

This is a RAW bacc kernel (no TileContext — Tile's fixed prologue/epilogue
dominates a kernel this small). Manual semaphores: one per input DMA
(a shared counter cannot order completions — the 16 SDMA engines
interleave chunks), s_act/s_pe/s_dve for cross-engine edges, with
engine DRAINs carrying the increments wherever a DMA reads an engine's
output next (the plain inc fires at instruction retire, before the
write pipeline drains). Seg chunks ride the SP HWDGE ring; rowhit/colhit
ride the gpsimd SWDGE ring in parallel. bf16 PE warm-up dummies raise the
PE pstate before the real matmuls. Outputs are raw per-partition partial
sums ([128,4] J sums + the [rb,640] mask product tile); the host finishes
both reductions.

Sharding: data-parallel over batch, 2 images per core on 8 cores (SPMD).
"""

import math
import numpy as np
import ml_dtypes

import concourse.bass as bass
import concourse.bacc as bacc
import concourse.mybir as mybir
from concourse.bass_utils import run_bass_kernel_spmd

WEIGHT = 0.1
B, M, H, W = 16, 256, 640, 640

# Keep only the sigmoid table set so a single ACT_TABLE_LOAD covers the
# kernel (indices must be preserved — act_func_set_id is the index into
# act_info.json).
_ACT_TABLE_KEEP = "sigmoid_and_others"
_orig_get_activation_tables = None


def _patch_act_tables():
    global _orig_get_activation_tables
    if _orig_get_activation_tables is not None:
        return
    import concourse.hw_specs as hw_specs
    _orig_get_activation_tables = hw_specs.get_activation_tables

    def patched(arch):
        tabs = _orig_get_activation_tables(arch)
        if _ACT_TABLE_KEEP in tabs:
            tabs = {name: (fns if name == _ACT_TABLE_KEEP else set())
                    for name, fns in tabs.items()}
        return tabs

    hw_specs.get_activation_tables = patched
    bacc.get_activation_tables = patched


N_CORES = 8
IPC = B // N_CORES          # images per core
PT = 128                    # SBUF partitions
NT = H // PT                # row tiles per image (5)
NWI = NT * W                # columns per image in SBUF layout (3200)
NW = IPC * NWI              # seg columns per core (6400)
NB = 4                      # sigmoid blocks
BW = NW // NB               # block width (1600)
KP = 32                     # rect slots per (image, chunk): psum partition band
SEG_NP = ml_dtypes.float8_e4m3
SEG_DT = mybir.dt.float8e4

# fast-log constants (bf16): value bits J = 128*E + m, E exponent, m mantissa
# ln p = ln2/128 * (J - 128*127) + ln2*(log2(1+m/128) - m/128)
# mean of the sawtooth term over uniform mantissa: C0 = 1.5 - 1/ln2
_C0 = 1.5 - 1.0 / math.log(2.0)
_KC = 16256.0 - 128.0 * _C0            # J offset incl. mean correction
_LN2_128 = math.log(2.0) / 128.0

_PROG_CACHE: dict[tuple, object] = {}

# test-harness hooks (ignored in normal use): set TRACE=True to profile the
# SPMD launch; the BassKernelResults lands in LAST_RESULT.
TRACE = False
LAST_RESULT = None


# ----------------------------------------------------------------- host prep

def _box_coords(bboxes: np.ndarray, h: int, w: int):
    """Integer box corners, bit-exact float32 math as the reference."""
    bb = bboxes.astype(np.float32)
    cx = bb[:, 0] * np.float32(w)
    cy = bb[:, 1] * np.float32(h)
    bw = bb[:, 2] * np.float32(w)
    bh = bb[:, 3] * np.float32(h)
    two = np.float32(2.0)
    x1 = np.clip(cx - bw / two, 0.0, w - 1).astype(np.int32)
    y1 = np.clip(cy - bh / two, 0.0, h - 1).astype(np.int32)
    x2 = np.clip(cx + bw / two, 0.0, w - 1).astype(np.int32)
    y2 = np.clip(cy + bh / two, 0.0, h - 1).astype(np.int32)
    return x1, y1, x2, y2


def _disjoint_rects(boxes):
    """boxes: list of (x1,y1,x2,y2) inclusive ints. Returns disjoint rects
    (x1,x2,y1,y2) inclusive whose union equals the union of the boxes."""
    if not boxes:
        return []
    edges = sorted(set([b[0] for b in boxes] + [b[2] + 1 for b in boxes]))
    slabs = []
    for i in range(len(edges) - 1):
        xs, xe = edges[i], edges[i + 1]
        active = sorted((b[1], b[3]) for b in boxes if b[0] <= xs and b[2] + 1 >= xe)
        ints = []
        for a, bb in active:
            if ints and a <= ints[-1][1] + 1:
                ints[-1][1] = max(ints[-1][1], bb)
            else:
                ints.append([a, bb])
        if ints:
            slabs.append((xs, xe, tuple(tuple(t) for t in ints)))
    merged = []
    for xs, xe, ints in slabs:
        if merged and merged[-1][1] == xs and merged[-1][2] == ints:
            merged[-1][1] = xe
        else:
            merged.append([xs, xe, ints])
    out = []
    for xs, xe, ints in merged:
        for a, bb in ints:
            out.append((xs, xe - 1, a, bb))
    return out


# ------------------------------------------------------------- device program

def _build_program(n_chunks: int):
    """SPMD raw-bass program for one core: IPC images, each with n_chunks
    groups of up to KP disjoint rects. Returns compiled Bacc."""
    R = IPC * n_chunks          # virtual (image, chunk) pairs
    G = (R + 3) // 4            # psum groups (4 bands of 32 per tile)
    RH = R * NT * KP            # rowhit cols
    CO = 4                      # output columns (one J sum per sigmoid block)
    _patch_act_tables()
    nc = bacc.Bacc("TRN2", target_bir_lowering=False, debug=False)

    blob = nc.dram_tensor("blob", [PT, NW + RH], SEG_DT, kind="ExternalInput")
    colh = nc.dram_tensor("colh", [PT, G * W], mybir.dt.bfloat16,
                          kind="ExternalInput")
    outv = nc.dram_tensor("outv", [PT, CO], mybir.dt.float32,
                          kind="ExternalOutput")
    outm = nc.dram_tensor("outm", [PT, G * W], mybir.dt.float32,
                          kind="ExternalOutput")

    AF = mybir.ActivationFunctionType
    OP = mybir.AluOpType
    U16 = mybir.dt.uint16
    F32 = mybir.dt.float32
    BF16 = mybir.dt.bfloat16
    XA = mybir.AxisListType

    with nc.cleanup_on_exit():
        # one semaphore per DMA: a shared counter cannot order completions
        # (the 16 SDMA engines interleave chunks)
        sc = [nc.alloc_semaphore(f"sc{k}") for k in range(4)]
        s_rh = nc.alloc_semaphore("s_rh")
        s_ch = nc.alloc_semaphore("s_ch")
        s_out = nc.alloc_semaphore("s_out")
        s_act = nc.alloc_semaphore("s_act")   # sigmoid blocks done
        s_pe = nc.alloc_semaphore("s_pe")     # all matmuls done
        s_dve = nc.alloc_semaphore("s_dve")   # bitreds + mask TTs done

        with (
            nc.sbuf_tensor("seg", [PT, NW], SEG_DT) as seg,
            nc.sbuf_tensor("rh", [PT, RH], SEG_DT) as rh,
            nc.sbuf_tensor("ch", [PT, G * W], BF16) as ch,
            nc.sbuf_tensor("sig", [PT, NW], BF16) as sig,
            nc.sbuf_tensor("p1", [PT, NW // 2], BF16) as p1,
            nc.sbuf_tensor("p2", [PT, NW // 4], BF16) as p2,
            nc.sbuf_tensor("p3", [PT, NW // 8], BF16) as p3,
            nc.sbuf_tensor("combo", [PT, CO], F32) as combo,
            nc.psum_tensor("pss", [PT, G * W], F32) as pss,
            nc.psum_tensor("dps", [KP, 512], F32) as dps,
            nc.sbuf_tensor("scr", [PT, G * W], F32) as scr,
        ):
            # ---- input DMAs. Image 0 streams on the SP ring; rowhit,
            #      image 1 and colhit stream in parallel on the gpsimd/SWDGE
            #      ring. Chunks align with sigmoid blocks.
            EDGE = [0, 1600, 3200, 4800, 6400]
            CHUNKS = [(0, 1600), (1600, 3200), (3200, 4800), (4800, 6400)]
            for k, (lo, hi) in enumerate(CHUNKS):
                nc.sync.dma_start(seg[:, lo:hi],
                                  blob[:, lo:hi]).then_inc(sc[k], 16)
            nc.gpsimd.dma_start(rh[:], blob[:, NW:NW + RH]).then_inc(s_rh, 16)
            nc.gpsimd.dma_start(ch[:], colh[:]).then_inc(s_ch, 16)

            # ---- ACT: dummy tiny ACTIVATE first so the auto-inserted
            #      ACT_TABLE_LOAD runs immediately (overlapping the DMA)
            #      instead of after the first data wait.
            nc.scalar.activation(sig[:, 0:8], seg[:, 0:8], AF.Sigmoid,
                                 scale=-1.0)
            blk_sem = [sc[0], sc[1], sc[2], sc[3]]
            NBL = len(EDGE) - 1
            for b in range(NBL):
                lo, hi = EDGE[b], EDGE[b + 1]
                nc.scalar.wait_ge(blk_sem[b], 16)
                nc.scalar.activation(sig[:, lo:hi], seg[:, lo:hi],
                                     AF.Sigmoid, scale=-1.0).then_inc(s_act, 1)

            # ---- PE: bf16 warm-up dummies raise the PE pstate before the
            #      real matmuls (p3 is not written until ~2us after these
            #      retire, so the garbage reads never race a writer).
            for _ in range(10):
                nc.tensor.matmul(dps[0:32, 0:512], p3[:, 0:32], p3[:, 0:512],
                                 start=True, stop=True)
            # mask row contraction into 32-partition bands of pss.
            nc.tensor.wait_ge(s_rh, 16)        # rowhit
            for v in range(R):
                i = v // n_chunks
                g, band = divmod(v, 4)
                po = band * KP
                if v % n_chunks == 0:          # first chunk of each image
                    for k in ([0, 1] if i == 0 else [2, 3]):
                        nc.tensor.wait_ge(sc[k], 16)
                for t in range(NT):
                    lhsT = rh[:, (v * NT + t) * KP:(v * NT + t + 1) * KP]
                    rhs = seg[:, i * NWI + t * W:i * NWI + (t + 1) * W]
                    mm = nc.tensor.matmul(
                        pss[po:po + KP, g * W:g * W + 512], lhsT, rhs[:, 0:512],
                        start=(t == 0), stop=(t == NT - 1))
                    mm2 = nc.tensor.matmul(
                        pss[po:po + KP, g * W + 512:(g + 1) * W], lhsT,
                        rhs[:, 512:W],
                        start=(t == 0), stop=(t == NT - 1))
            del mm, mm2
            nc.tensor.drain().then_inc(s_pe, 1)   # psum writes landed

            # ---- DVE: per-block product tree + J bit sums; mask multiply
            #      interleaved where PE/colhit are ready and DVE has slack.
            off = [0, 0, 0]     # running p1/p2/p3 offsets

            def chain(b):
                lo, hi = EDGE[b], EDGE[b + 1]
                h = (hi - lo) // 2
                q = (hi - lo) // 4
                e = (hi - lo) // 8
                o1, o2, o3 = off
                off[0] += h
                off[1] += q
                off[2] += e
                nc.vector.wait_ge(s_act, b + 1)
                nc.vector.tensor_tensor(
                    p1[:, o1:o1 + h], sig[:, lo:lo + h],
                    sig[:, lo + h:hi], op=OP.mult)
                nc.vector.tensor_tensor(
                    p2[:, o2:o2 + q], p1[:, o1:o1 + q],
                    p1[:, o1 + q:o1 + h], op=OP.mult)
                nc.vector.tensor_tensor(
                    p3[:, o3:o3 + e], p2[:, o2:o2 + e],
                    p2[:, o2 + e:o2 + q], op=OP.mult)
                with nc.allow_low_precision(reason="u16 bit sum in f32"):
                    red = nc.vector.tensor_reduce(
                        combo[:, b:b + 1],
                        p3[:, o3:o3 + e].bitcast(U16),
                        axis=XA.X, op=OP.add)
                if b == NBL - 1:
                    # last writer before the out DMA: drain the DVE pipe so
                    # the combo write has landed before the semaphore fires
                    nc.vector.drain().then_inc(s_dve, 1)
                else:
                    red.then_inc(s_dve, 1)

            chain(0)
            chain(1)
            chain(2)
            # mask multiply (PSUM x colhit -> scr, f32); host sums via outm.
            # Sitting here it also buffers the last block's chain from its
            # ACTIVATE by ~700ns.
            nc.vector.wait_ge(s_pe, 1)
            nc.vector.wait_ge(s_ch, 16)
            nc.vector.tensor_tensor(scr[:], pss[:], ch[:],
                                    op=OP.mult).then_inc(s_dve, 1)
            chain(3)

            # ---- out: mask product tile streams while the last blocks
            #      finish; combo follows after the last bitred.
            rb = min(R, 4) * KP          # rows carrying mask bands
            nc.sync.wait_ge(s_dve, 4)
            nc.sync.dma_start(outm[0:rb, :], scr[0:rb, :]).then_inc(s_out, 16)
            nc.sync.wait_ge(s_dve, 5)
            nc.sync.dma_start(outv[:], combo[:]).then_inc(s_out, 16)
            nc.sync.wait_ge(s_out, 32)
        nc.all_engine_barrier()

    nc.compile()
    return nc


def _get_program(n_chunks: int):
    if n_chunks not in _PROG_CACHE:
        _PROG_CACHE[n_chunks] = _build_program(n_chunks)
    return _PROG_CACHE[n_chunks]


# -------------------------------------------------------------------- kernel

def kernel(seg_pred: np.ndarray, bboxes: np.ndarray, batch_idx: np.ndarray,
           is_seg: np.ndarray) -> np.ndarray:
    seg_pred = np.asarray(seg_pred, dtype=np.float32)
    bboxes = np.asarray(bboxes, dtype=np.float32)
    batch_idx = np.asarray(batch_idx)
    is_seg = np.asarray(is_seg).astype(bool)
    assert seg_pred.shape == (B, 1, H, W), seg_pred.shape

    x1, y1, x2, y2 = _box_coords(bboxes, H, W)
    per_img = [[] for _ in range(B)]
    has_box = np.zeros(B, dtype=bool)
    for m in range(bboxes.shape[0]):
        bi = int(batch_idx[m])
        has_box[min(max(bi, 0), B - 1)] = True   # reference clips for has_box
        if 0 <= bi < B:
            per_img[bi].append((int(x1[m]), int(y1[m]), int(x2[m]), int(y2[m])))

    eff = (~is_seg) & has_box
    if not (eff.any() and not is_seg.all()):
        return np.float32(0.0)

    rects = [_disjoint_rects(p) if e else [] for p, e in zip(per_img, eff)]
    k_max = max((len(r) for r in rects), default=0)
    n_chunks = max(1, math.ceil(k_max / KP))
    R = IPC * n_chunks
    G = (R + 3) // 4
    RH = R * NT * KP

    in_maps = []
    for core in range(N_CORES):
        imgs = [core * IPC + i for i in range(IPC)]
        blob = np.zeros((PT, NW + RH), SEG_NP)
        colh = np.zeros((PT, G * W), ml_dtypes.bfloat16)
        for i, b in enumerate(imgs):
            if eff[b]:
                # [p, t*W + x] layout: partition payload contiguous per image
                blob[:, i * NWI:(i + 1) * NWI] = (
                    seg_pred[b, 0].reshape(NT, PT, W).transpose(1, 0, 2)
                    .reshape(PT, NWI).astype(SEG_NP))
            for r, (rx1, rx2, ry1, ry2) in enumerate(rects[b]):
                c, rr = divmod(r, KP)
                v = i * n_chunks + c
                g, band = divmod(v, 4)
                colh[band * KP + rr, g * W + rx1:g * W + rx2 + 1] = 1
                for t in range(NT):
                    lo, hi = max(ry1, t * PT), min(ry2, t * PT + PT - 1)
                    if lo <= hi:
                        col = NW + (v * NT + t) * KP + rr
                        blob[lo - t * PT:hi - t * PT + 1, col] = 1
        in_maps.append({"blob": blob, "colh": colh})

    nc = _get_program(n_chunks)
    global LAST_RESULT
    res = run_bass_kernel_spmd(nc, in_maps, list(range(N_CORES)), trace=TRACE)
    LAST_RESULT = res

    # host reduction in float64
    n_comp = PT * (NW // 8)     # compressed J elements per core
    total = 0.0
    for core in range(N_CORES):
        jsum = res.results[core]["outv"].astype(np.float64).sum()
        om = res.results[core]["outm"]
        msum = 0.0
        for g in range(G):
            rg = min(R - g * 4, 4) * KP
            msum += om[0:rg, g * W:(g + 1) * W].astype(np.float64).sum()
        total += -(jsum - n_comp * _KC) * _LN2_128 - msum
    loss = WEIGHT * total / (B * H * W)
    return np.float32(loss)
